# revision 6
# baseline (speedup 1.0000x reference)
"""Trainium2 Bass kernel for block-causal sparse attention (MLA-style KV).

Sharding: tensor-parallel over heads. 16 heads / 8 cores = 2 heads per core,
one KV head per core-pair. Each core computes q/k/v projections from the full
(transposed) x, RoPE, sparse attention for its 2 heads, and a partial output
projection; the host sums the 8 partial outputs.

Sparsity structure (T=4096, BLOCK=128, WINDOW=512, GLOBAL_EVERY=64):
for query block b, visible keys are blocks b-4..b (block b-4 masked by a fixed
triangular+global pattern) plus "global" columns j%64==0 with j < 128*(b-4).

All tensors move as bfloat16 (halves HBM traffic; matmuls run 1 cycle/row).
DRAM tensors are pre-packed on the host into their exact SBUF layouts so each
needs only one (or a few chunked) DMA — the HWDGE descriptor-generation queue
(625 ns per DMA instruction) would otherwise serialize the kernel.
Scores are computed transposed ([k, q] layout) so probabilities feed the PV
and output-projection matmuls with no transposes. Softmax denominators are
computed with ones-matmul partition reductions accumulated in PSUM; the
reciprocal is broadcast across partitions on GPSIMD. The output projection
of tile i is emitted between the attention items of tile i+1 so the PE has
filler work while exp/mask latency drains.
"""

import numpy as np

N_CORES = 8
T = 4096
C = 2048
L = 512
H = 16
KVH = 4
HD = 128
BLOCK = 128
WINDOW = 512
GLOBAL_EVERY = 64
ROPE_THETA = 10000.0

QTW = 512            # query tile width (4 blocks)
NQT = T // QTW       # 8
NKT = C // 128       # 16 contraction tiles for projections
NG = T // GLOBAL_EVERY  # 64 global columns

_CACHE = {}


def _build_module():
    import concourse.bacc as bacc
    import concourse.mybir as mybir
    import concourse.tile as tile
    from contextlib import ExitStack

    F32 = mybir.dt.float32
    BF = mybir.dt.bfloat16
    EXP = mybir.ActivationFunctionType.Exp

    nc = bacc.Bacc("TRN2", target_bir_lowering=False, debug=False,
                   num_devices=N_CORES)

    # all DRAM tensors pre-packed host-side into SBUF layout [128, flat]
    xtd = nc.dram_tensor("xtd", [NQT, 128, NKT * 512], BF, kind="ExternalInput")
    wqd = nc.dram_tensor("wqd", [128, NKT * 256], BF, kind="ExternalInput")
    wkvd = nc.dram_tensor("wkvd", [128, NKT * 256], BF, kind="ExternalInput")
    wod = nc.dram_tensor("wod", [128, 2 * C], BF, kind="ExternalInput")
    cosd = nc.dram_tensor("cosd", [HD, T], BF, kind="ExternalInput")
    sind = nc.dram_tensor("sind", [HD, T], BF, kind="ExternalInput")  # sign-folded
    maskt = nc.dram_tensor("maskt", [128, 128], BF, kind="ExternalInput")
    maskg = nc.dram_tensor("maskg", [NG, T], BF, kind="ExternalInput")
    onesd = nc.dram_tensor("onesd", [128, 1], BF, kind="ExternalInput")
    identd = nc.dram_tensor("identd", [128, 128], BF, kind="ExternalInput")
    permd = nc.dram_tensor("permd", [128, 128], BF, kind="ExternalInput")
    out = nc.dram_tensor("out", [T, C], BF, kind="ExternalOutput")

    scale = 1.0 / np.sqrt(HD)

    with tile.TileContext(nc) as tc, ExitStack() as ctx:
        res = ctx.enter_context(tc.tile_pool(name="res", bufs=1))
        kT = res.tile([128, T], BF, tag="kT")
        vN = res.tile([128, T], BF, tag="vN")
        kG = res.tile([128, NG], BF, tag="kG")
        vG = res.tile([64, 128], BF, tag="vG")
        vGT = res.tile([128, NG], BF, tag="vGT")
        mT = res.tile([128, 128], BF, tag="mT")
        mG = res.tile([NG, T], BF, tag="mG")
        ones = res.tile([128, 1], BF, tag="ones")
        ident = res.tile([128, 128], BF, tag="ident")
        perm = res.tile([128, 128], BF, tag="perm")
        wo_sb = res.tile([128, 2 * C], BF, tag="wo_sb")
        wq_sb = res.tile([128, NKT * 256], BF, tag="wq_sb")
        wkv_sb = res.tile([128, NKT * 256], BF, tag="wkv_sb")
        cos_sb = res.tile([128, T], BF, tag="cos_sb")
        sin_sb = res.tile([128, T], BF, tag="sin_sb")

        xpool = ctx.enter_context(tc.tile_pool(name="xpool", bufs=2))
        qlp = ctx.enter_context(tc.tile_pool(name="qlp", bufs=2))
        vtp = ctx.enter_context(tc.tile_pool(name="vtp", bufs=2))
        swp = ctx.enter_context(tc.tile_pool(name="swp", bufs=2))
        ppool = ctx.enter_context(tc.tile_pool(name="ppool", bufs=3))
        ynp = ctx.enter_context(tc.tile_pool(name="ynp", bufs=2))
        recp = ctx.enter_context(tc.tile_pool(name="recp", bufs=2))
        rbcp = ctx.enter_context(tc.tile_pool(name="rbcp", bufs=2))
        obp = ctx.enter_context(tc.tile_pool(name="obp", bufs=2))

        # pj (projection) and s (score) tiles share one PSUM pool: their
        # lifetimes alternate within a tile, and attention gets 4-deep
        # score lookahead.
        psp = ctx.enter_context(tc.tile_pool(name="psp", bufs=4, space="PSUM"))
        ypool = ctx.enter_context(tc.tile_pool(name="ypool", bufs=1, space="PSUM"))
        dpool = ctx.enter_context(tc.tile_pool(name="dpool", bufs=1, space="PSUM"))
        opool = ctx.enter_context(tc.tile_pool(name="opool", bufs=2, space="PSUM"))

        def make_wo_blocks(ynorm, qs0):
            """Return a list of 16 closures, each emitting one [128,512]
            output-projection block (2 matmuls + copy); every 4th block
            also stores the completed [128, 2048] output row-band."""
            obs = {}
            blocks = []
            for qs in range(4):
                for n in range(4):
                    def emit(qs=qs, n=n):
                        if n == 0:
                            obs[qs] = obp.tile([128, C], BF, tag="ob",
                                               name="ob")
                        o_ps = opool.tile([128, 512], F32, tag="o", name="o_ps")
                        nc.tensor.matmul(o_ps[:], ynorm[0][:, qs * 128:(qs + 1) * 128],
                                         wo_sb[:, n * 512:n * 512 + 512],
                                         start=True, stop=False)
                        nc.tensor.matmul(o_ps[:], ynorm[1][:, qs * 128:(qs + 1) * 128],
                                         wo_sb[:, C + n * 512:C + n * 512 + 512],
                                         start=False, stop=True)
                        ob = obs[qs]
                        eng = (qs * 4 + n) % 3
                        if eng == 0:
                            nc.scalar.copy(ob[:, n * 512:(n + 1) * 512], o_ps[:])
                        elif eng == 1:
                            nc.gpsimd.tensor_copy(ob[:, n * 512:(n + 1) * 512],
                                                  o_ps[:])
                        else:
                            nc.vector.tensor_copy(ob[:, n * 512:(n + 1) * 512],
                                                  o_ps[:])
                        if n == 3:
                            rows = slice(qs0 + qs * 128, qs0 + (qs + 1) * 128)
                            nc.sync.dma_start(out[rows, :], ob[:])
                    blocks.append(emit)
            return blocks

        pending_wo = []
        for it in range(NQT):
            nt = it
            b0 = 4 * it
            ts = slice(nt * 512, (nt + 1) * 512)
            qs0 = it * QTW

            # ---- projections for t-tile `nt` ----
            xbig = xpool.tile([128, NKT * 512], BF, tag="xtile")
            if it == 0:
                # chunked loads so the first matmuls chase the DMA stream;
                # four concurrently-open PSUM accumulation groups cycle
                # per contraction tile
                kt0 = 0
                for nk in (1, 1, 2, 4, 4, 4):
                    cs = slice(kt0 * 256, (kt0 + nk) * 256)
                    nc.sync.dma_start(wq_sb[:, cs], wqd[:, cs])
                    nc.sync.dma_start(wkv_sb[:, cs], wkvd[:, cs])
                    xs = slice(kt0 * 512, (kt0 + nk) * 512)
                    nc.sync.dma_start(xbig[:, xs], xtd[0, :, xs])
                    if kt0 == 0:
                        nc.gpsimd.dma_start(ident[:], identd[:])
                        nc.gpsimd.dma_start(perm[:], permd[:])
                        nc.gpsimd.dma_start(mT[:], maskt[:])
                        nc.gpsimd.dma_start(ones[:], onesd[:])
                        nc.sync.dma_start(cos_sb[:, 0:512], cosd[:, 0:512])
                        nc.sync.dma_start(sin_sb[:, 0:512], sind[:, 0:512])
                    kt0 += nk
                nc.sync.dma_start(cos_sb[:, 512:], cosd[:, 512:])
                nc.sync.dma_start(sin_sb[:, 512:], sind[:, 512:])
            else:
                # quarter-chunked prefetch so small DMAs never queue long
                # behind a big transfer on the serial DMA engines
                for ck in range(4):
                    xs = slice(ck * 2048, (ck + 1) * 2048)
                    nc.sync.dma_start(xbig[:, xs], xtd[nt, :, xs])

            qloc = [qlp.tile([128, 512], BF, tag=f"ql{h}", name=f"ql{h}")
                    for h in range(2)]
            wslices = [
                lambda kt: wq_sb[:, kt * 256:kt * 256 + 128],
                lambda kt: wq_sb[:, kt * 256 + 128:kt * 256 + 256],
                lambda kt: wkv_sb[:, kt * 256:kt * 256 + 128],
                lambda kt: wkv_sb[:, kt * 256 + 128:kt * 256 + 256],
            ]
            vT_t = vtp.tile([128, 512], BF, tag="vT")
            pjs = [psp.tile([128, 512], F32, tag="ps", name=f"pj{i}")
                   for i in range(4)]
            if it == 0:
                # interleave passes per kt so matmuls chase the DMA stream
                for kt in range(NKT):
                    for i in range(4):
                        nc.tensor.matmul(pjs[i][:], wslices[i](kt),
                                         xbig[:, kt * 512:(kt + 1) * 512],
                                         start=(kt == 0), stop=(kt == NKT - 1))
            else:
                for i in range(4):
                    for kt in range(NKT):
                        nc.tensor.matmul(pjs[i][:], wslices[i](kt),
                                         xbig[:, kt * 512:(kt + 1) * 512],
                                         start=(kt == 0), stop=(kt == NKT - 1))
            for i in range(4):
                pj = pjs[i]
                if i < 3:
                    # RoPE: dest = pj*cos + swap(pj)*sinS
                    dest = qloc[i][:] if i < 2 else kT[:, ts]
                    qsb = swp.tile([128, 512], BF, tag="qsb")
                    nc.scalar.copy(qsb[:], pj[:])
                    sw = psp.tile([128, 512], F32, tag="ps", name="sw")
                    nc.tensor.matmul(sw[:], perm[:], qsb[:],
                                     start=True, stop=True)
                    ta = swp.tile([128, 512], F32, tag="ta")
                    nc.vector.tensor_mul(ta[:], pj[:], cos_sb[:, ts])
                    tb = swp.tile([128, 512], F32, tag="tb")
                    nc.vector.tensor_mul(tb[:], sw[:], sin_sb[:, ts])
                    nc.vector.tensor_add(dest, ta[:], tb[:])
                else:
                    nc.vector.tensor_copy(vT_t[:], pj[:])

            if it == 0:
                nc.gpsimd.dma_start(mG[:], maskg[:])
                nc.sync.dma_start(wo_sb[:], wod[:, :])

            # ---- v transpose for this t-tile + incremental global K/V ----
            for j in range(4):
                blk = nt * 4 + j
                tp = psp.tile([128, 128], BF, tag="ps", name="tp")
                nc.tensor.transpose(tp[:, :128], vT_t[:, j * 128:(j + 1) * 128],
                                    ident[:])
                nc.scalar.copy(vN[:, blk * 128:(blk + 1) * 128], tp[:, :128])
            gsl = slice(nt * 8, (nt + 1) * 8)
            nc.vector.tensor_copy(kG[:, gsl], kT[:, ts][:, 0:512:GLOBAL_EVERY])
            nc.vector.tensor_copy(vGT[:, gsl], vT_t[:][:, 0:512:GLOBAL_EVERY])
            gw2 = 8 * (nt + 1)
            tpg = psp.tile([128, 128], BF, tag="ps", name="tpg")
            nc.tensor.transpose(tpg[:gw2, :128], vGT[:, :gw2], ident[:])
            nc.vector.tensor_copy(vG[:gw2, :], tpg[:gw2, :128])

            # ---- attention for query tile `it` (4 blocks b0..b0+3), with
            # the previous tile's output projection interleaved ----
            gw = min(NG, 8 * it)   # written prefix of kG/vG; 0 for it=0
            ynorm = []
            for h in range(2):
                items = [(b0, 0, 512, None)]
                if it == 0:
                    for j in range(3):
                        items.append((j + 1, (j + 1) * 128, (3 - j) * 128, None))
                    use_glob = False
                else:
                    for j in range(4):
                        items.append((b0 - 4 + j, 0, (j + 1) * 128, j))
                    for j in range(3):
                        items.append((b0 + 1 + j, (j + 1) * 128, (3 - j) * 128, None))
                    use_glob = gw > 0

                y_ps = ypool.tile([128, QTW], F32, tag="y")
                d_ps = dpool.tile([1, QTW], F32, tag="d")
                n_items = len(items) + (1 if use_glob else 0)
                s_tiles = [None] * n_items

                def emit_qk(ii):
                    s = psp.tile([128, QTW], F32, tag="ps")
                    if ii < len(items):
                        kb, qoff, w, _ = items[ii]
                        nc.tensor.matmul(
                            s[:, :w], kT[:, kb * 128:(kb + 1) * 128],
                            qloc[h][:, qoff:qoff + w],
                            start=True, stop=True)
                    else:
                        nc.tensor.matmul(s[:gw, :], kG[:, :gw], qloc[h][:],
                                         start=True, stop=True)
                    s_tiles[ii] = s

                def emit_rest(ii):
                    first = ii == 0
                    last = ii == n_items - 1
                    s = s_tiles[ii]
                    p = ppool.tile([128, QTW], BF, tag="p")
                    if ii < len(items):
                        kb, qoff, w, tri = items[ii]
                        nc.scalar.activation(p[:, :w], s[:, :w], EXP, scale=scale)
                        if tri is not None:
                            nc.vector.tensor_mul(p[:, tri * 128:(tri + 1) * 128],
                                                 p[:, tri * 128:(tri + 1) * 128],
                                                 mT[:])
                        nc.tensor.matmul(y_ps[:, qoff:qoff + w],
                                         vN[:, kb * 128:(kb + 1) * 128], p[:, :w],
                                         start=first, stop=last)
                        nc.tensor.matmul(d_ps[:, qoff:qoff + w], ones[:, :],
                                         p[:, :w], start=first, stop=last)
                    else:
                        nc.scalar.activation(p[:gw, :], s[:gw, :], EXP, scale=scale)
                        nc.vector.tensor_mul(p[:gw, :], p[:gw, :],
                                             mG[:gw, qs0:qs0 + QTW])
                        nc.tensor.matmul(y_ps[:, :], vG[:gw, :], p[:gw, :],
                                         start=first, stop=last)
                        nc.tensor.matmul(d_ps[:, :], ones[:gw, :], p[:gw, :],
                                         start=first, stop=last)

                emit_qk(0)
                for ii in range(n_items):
                    if ii + 1 < n_items:
                        emit_qk(ii + 1)
                    if pending_wo:
                        pending_wo.pop(0)()
                    emit_rest(ii)

                rec = recp.tile([1, QTW], F32, tag="rec")
                nc.vector.reciprocal(rec[:], d_ps[:])
                rbc = rbcp.tile([128, QTW], F32, tag="rbc")
                nc.gpsimd.partition_broadcast(rbc[:], rec[:])
                yn = ynp.tile([128, QTW], BF, tag=f"yn{h}", name=f"yn{h}")
                nc.vector.tensor_mul(yn[:], y_ps[:], rbc[:])
                ynorm.append(yn)

            # any leftover blocks from the previous tile (it=1 only)
            while pending_wo:
                pending_wo.pop(0)()
            # ---- output projection: deferred into the next tile's
            # attention items (PE filler while exp latency drains) ----
            pending_wo = make_wo_blocks(ynorm, qs0)

        while pending_wo:
            pending_wo.pop(0)()

    nc.compile()
    return nc


def _host_inputs(x, w_q, w_kv_down, w_k_up, w_v_up, w_o):
    """Build the per-core input maps (host-side shard + precompute).

    Weight/x tensors are packed into their SBUF layouts ([128 partitions,
    flattened contraction-tile x column]) so the device needs one DMA each.
    """
    import ml_dtypes
    BF = ml_dtypes.bfloat16

    x = np.asarray(x)
    w_q = np.asarray(w_q)
    w_kv_down = np.asarray(w_kv_down)
    w_k_up = np.asarray(w_k_up)
    w_v_up = np.asarray(w_v_up)
    w_o = np.asarray(w_o)
    x2 = np.ascontiguousarray(x.reshape(T, C).astype(np.float32))
    xt = x2.T.astype(BF)                                             # [C, T]
    # pack to [nt, p, kt*512 + c]
    xtd = np.ascontiguousarray(
        xt.reshape(NKT, 128, NQT, 512).transpose(2, 1, 0, 3)
        .reshape(NQT, 128, NKT * 512))

    def pack_w(w):  # [C, F] -> [128, NKT*F]
        Fc = w.shape[1]
        return np.ascontiguousarray(
            np.asarray(w).reshape(NKT, 128, Fc).transpose(1, 0, 2)
            .reshape(128, NKT * Fc).astype(BF))

    # RoPE tables, [hd, t] layout, sign folded into sin for the swapped term
    freqs = 1.0 / (ROPE_THETA ** (np.arange(0, HD, 2, dtype=np.float64) / HD))
    emb = np.arange(T, dtype=np.float64)[:, None] * freqs[None, :]   # [T, 64]
    cos = np.concatenate([np.cos(emb), np.cos(emb)], axis=-1)        # [T, 128]
    sin = np.concatenate([np.sin(emb), np.sin(emb)], axis=-1)
    cosT = np.ascontiguousarray(cos.T.astype(BF))                    # [128, T]
    sinS = np.ascontiguousarray(sin.T.astype(BF))

    # fixed triangular+global mask for the b-4 key block, [k_off, q_off]
    oi = np.arange(128)
    mTm = ((oi[None, :] <= oi[:, None]) | (oi[:, None] % 64 == 0)).astype(BF)

    # global-column mask [g, q]: visible iff 64 g < 128 (q//128 - 4)
    g = np.arange(NG)
    qb = np.arange(T) // BLOCK
    mGm = (64 * g[:, None] < 128 * (qb[None, :] - 4)).astype(BF)

    onesv = np.ones((128, 1), BF)
    identv = np.eye(128, dtype=BF)
    # signed half-swap: sw[d] = -pj[d+64] for d<64, +pj[d-64] for d>=64.
    # used as matmul stationary [contraction d_in, out d]: perm[i, j] with
    # sw_j = sum_i perm[i, j] * pj_i
    permv = np.zeros((128, 128), np.float32)
    for jj in range(64):
        permv[jj + 64, jj] = -1.0
        permv[jj, jj + 64] = 1.0
    permv = permv.astype(BF)

    wk_f = (w_kv_down.astype(np.float32) @ w_k_up.astype(np.float32))  # [C, KVH*HD]
    wv_f = (w_kv_down.astype(np.float32) @ w_v_up.astype(np.float32))

    in_maps = []
    for c in range(N_CORES):
        h0 = 2 * c
        kv = h0 // (H // KVH)
        wqd_c = pack_w(w_q[:, h0 * HD:(h0 + 2) * HD].astype(np.float32))
        wkvd_c = pack_w(np.concatenate([
            wk_f[:, kv * HD:(kv + 1) * HD],
            wv_f[:, kv * HD:(kv + 1) * HD]], axis=1))
        wo_c = w_o[h0 * HD:(h0 + 2) * HD, :].astype(np.float32)       # [256, C]
        wod_c = np.ascontiguousarray(
            wo_c.reshape(2, 128, C).transpose(1, 0, 2).reshape(128, 2 * C)
            .astype(BF))
        in_maps.append({
            "xtd": xtd, "wqd": wqd_c, "wkvd": wkvd_c, "wod": wod_c,
            "cosd": cosT, "sind": sinS, "maskt": mTm, "maskg": mGm,
            "onesd": onesv, "identd": identv, "permd": permv,
        })
    return in_maps


def _get_module():
    if "nc" not in _CACHE:
        _CACHE["nc"] = _build_module()
    return _CACHE["nc"]


def kernel(x, w_q, w_kv_down, w_k_up, w_v_up, w_o):
    from concourse.bass_utils import run_bass_kernel_spmd

    nc = _get_module()
    in_maps = _host_inputs(x, w_q, w_kv_down, w_k_up, w_v_up, w_o)
    res = run_bass_kernel_spmd(nc, in_maps, list(range(N_CORES)))
    acc = np.zeros((T, C), np.float32)
    for c in range(N_CORES):
        acc += res.results[c]["out"].astype(np.float32)
    return acc.reshape(1, T, C)
